# revision 14
# baseline (speedup 1.0000x reference)
"""LIF neuron (leaky integrate, bidirectional threshold fire, hard reset)
on 8 Trainium2 NeuronCores.

Math (per element, recurrence over T):
    u      = V*(1 - 1/tau) + x_t        (tau = 5/3  =>  decay ~= 0.4)
    out_t  = (u >= 1) - (u <= -1)               in {-1, 0, +1}
    V'     = u * (|u| < 1)                      (hard reset to 0)

Sharding: data-parallel over batch (axis 1), B=32 -> 4 per core; the
recurrence is only over T and elementwise over B,C,H,W, so no
communication is needed.

Engine split (the baseline ran the integrate as a DVE stt at 1x fp32
rate, saturating DVE at ~69us and ACT at ~66us while PE sat idle):
  PE   u = (decay*I)^T V + I^T x_hi + I^T x_lo    three accumulating
       identity matmuls per 512-col PSUM bank.  x is Dekker-split on
       the host into two bf16 halves (x = hi + lo exactly to 2^-18
       rel), so the x path runs at full PE rate with fp32-grade
       precision and the SAME HBM bytes as one fp32 tensor.  The V
       path uses fp32r (tf32-like, ~2^-12): V' is bounded by 1 and
       only survives one decay step, so the extra ~2e-5 error in u
       flips only O(1e3) of the 10.7M spikes (rel err ~1e-2 < 2e-2).
  ACT  a  = |u|                        activation(Abs), PSUM source
  DVE  V' = (a is_lt 1) mult u         scalar_tensor_tensor -> fp32r
  enc  s8: groups 0-1 on ACT  sat_i8(relu(62.5*u + 63)),
           groups 2-3 on DVE  sat_i8(max(u,-1)*63.5)
Host decode: ACT groups  spike = (s8 >= 126) - (s8 == 0)
             DVE groups  spike = (s8 >= 64) - (s8 <= -64)
HBM traffic per core: 16.8 MB in + 4.2 MB out.
"""

import numpy as np

import concourse.bass as bass
import concourse.tile as tile
from concourse import mybir
from concourse.alu_op_type import AluOpType
from concourse.bass_utils import run_bass_kernel_spmd

try:
    import ml_dtypes
    _BF16 = ml_dtypes.bfloat16
except ImportError:  # pragma: no cover
    _BF16 = None


def _split_sync_waits(nc):
    """This walrus build enforces the ISA limit of one sync wait per
    instruction (two for EventSemaphore), but Tile's sem-assigner freely
    attaches several. Hoist excess waits onto NoOps inserted just before the
    offending instruction on the same engine (waits are monotonic sem-ge, so
    order among them is irrelevant)."""
    ctr = 0
    for f in nc.m.functions:
        for bb in f.blocks:
            il = bb.instructions
            i = 0
            while i < len(il):
                inst = il[i]
                si = getattr(inst, "sync_info", None)
                if si is not None:
                    lim = 2 if isinstance(inst, mybir.InstEventSemaphore) else 1
                    waits = list(si.on_wait)
                    if len(waits) > lim:
                        inst.sync_info = mybir.SyncInfo(
                            on_wait=waits[:lim], on_update=list(si.on_update))
                        for w in waits[lim:]:
                            ctr += 1
                            nop = mybir.InstNoOp(
                                name=f"I-wsplit-{ctr}",
                                engine=inst.engine,
                                bass_nofuse=True,
                                sync_info=mybir.SyncInfo(
                                    on_wait=[w], on_update=[]),
                            )
                            nc.register_instruction(nop, overwrite=True)
                            il.insert(i, nop)
                            i += 1
                i += 1
    return ctr


# ---------------------------------------------------------------------------
# Problem shape (hardcoded per spec: x [T, B, C, H, W] = [8, 32, 128, 32, 32])
T, B, C, H, W = 8, 32, 128, 32, 32
HW = H * W                      # 1024
N_CORES = 8
BS = B // N_CORES               # 4 batches per core
F = BS * HW                     # 4096 free-dim elements per step
G = 4                           # pipeline groups per step (1 batch each)
GF = F // G                     # 1024 (= 2 PSUM banks)
# fp32r(0.4): nearest tf32-grid value to the reference decay; hardcoded so
# host, CoreSim and hardware all use the identical weight
DECAY_R = 0.4000244140625  # fp32r(0.4): V path runs in fp32r

ENC_SCALE = 62.5
ENC_BIAS = 63.0

F32 = mybir.dt.float32
F32R = mybir.dt.float32r
BF16 = mybir.dt.bfloat16
I8 = mybir.dt.int8
ABS = mybir.ActivationFunctionType.Abs
RELU = mybir.ActivationFunctionType.Relu

_NC_CACHE = {}


def _build():
    if "nc" in _NC_CACHE:
        return _NC_CACHE["nc"]
    nc = bass.Bass()
    # hi/lo bf16 halves packed per group: xhl[t, g] = [C, 2*GF] with
    # columns [0:GF] = bf16 hi, [GF:2*GF] = bf16 lo -> one DMA per group
    xhl = nc.declare_dram_parameter("xhl", [T, G, C, 2 * GF], BF16,
                                    isOutput=False)
    eye = nc.declare_dram_parameter("eye", [C, C], BF16, isOutput=False)
    eyed = nc.declare_dram_parameter("eyed", [C, C], F32R, isOutput=False)
    out_s = nc.declare_dram_parameter("out_s", [T, C, F], I8, isOutput=True)

    with tile.TileContext(nc) as tc:
        with (
            tc.tile_pool(name="xp", bufs=6) as xp,
            tc.tile_pool(name="vp", bufs=8) as vp,
            tc.tile_pool(name="ap", bufs=4) as ap,
            tc.tile_pool(name="ep", bufs=3) as ep,
            tc.tile_pool(name="wp", bufs=1) as wp,
            tc.tile_pool(name="pp", bufs=4,
                         space=bass.MemorySpace.PSUM) as pp,
        ):
            bt = wp.tile([C, 1], F32, tag="bias")
            nc.vector.memset(bt[:], ENC_BIAS)
            eye_t = wp.tile([C, C], BF16, tag="eye")
            nc.sync.dma_start(out=eye_t[:], in_=eye[:])
            eyed_t = wp.tile([C, C], F32R, tag="eyed")
            nc.sync.dma_start(out=eyed_t[:], in_=eyed[:])
            # preload the ACT table so the first real activation doesn't
            # pay the ~2.7us table load on the critical path
            warm = wp.tile([C, 1], F32, tag="warm")
            nc.scalar.activation(warm[:], bt[:], ABS)
            nc.scalar.activation(warm[:], bt[:], RELU)

            state = [None] * G
            for t in range(T):
                xts = []
                for g in range(G):
                    xg = xp.tile([C, 2 * GF], BF16, tag="xg")
                    nc.sync.dma_start(out=xg[:], in_=xhl[t][g])
                    xts.append(xg)
                # per group: x matmuls (DMA-gated only) then the V matmul
                # closing the bank, so each PSUM group completes ~2.5us
                # after the previous one and its abs/reset/encode chain
                # starts immediately instead of in an end-of-step burst
                ps = []
                for g in range(G):
                    p = pp.tile([C, GF], F32, tag="p")
                    xg = xts[g]
                    for h in range(2):
                        sl = slice(h * 512, (h + 1) * 512)
                        nc.tensor.matmul(
                            p[:, sl], eye_t[:],
                            xg[:, h * 512:(h + 1) * 512],
                            start=True, stop=False)
                        nc.tensor.matmul(
                            p[:, sl], eye_t[:],
                            xg[:, GF + h * 512:GF + (h + 1) * 512],
                            start=False, stop=(t == 0))
                    if t > 0:
                        for h in range(2):
                            sl = slice(h * 512, (h + 1) * 512)
                            nc.tensor.matmul(
                                p[:, sl], eyed_t[:], state[g][:, sl],
                                start=False, stop=True)
                    ps.append(p)
                # per group: abs -> reset -> encode, so each PSUM bank pair
                # frees as early as possible (the next step's x matmuls are
                # queued on the PE waiting for it, and a >3us PE gap would
                # re-throttle the HAM to half rate)
                e = ep.tile([C, F], I8)
                for g in range(G):
                    sl = slice(g * GF, (g + 1) * GF)
                    if t < T - 1:
                        a = ap.tile([C, GF], F32)
                        nc.scalar.activation(a[:], ps[g][:], ABS)
                        v = vp.tile([C, GF], F32R, tag="v")
                        nc.vector.scalar_tensor_tensor(
                            v[:], a[:], 1.0, ps[g][:],
                            AluOpType.is_lt, AluOpType.mult)
                        state[g] = v
                    if g < 2:
                        nc.scalar.activation(e[:, sl], ps[g][:], RELU,
                                             bias=bt[:], scale=ENC_SCALE)
                    else:
                        nc.vector.tensor_scalar(
                            e[:, sl], ps[g][:], -1.0, 63.5,
                            AluOpType.max, AluOpType.mult)
                # store via the (otherwise idle) GpSimd queue: its issue
                # waits on the encodes, and on the Sync queue it would block
                # the next step's loads behind that wait
                nc.gpsimd.dma_start(out=out_s[t][:], in_=e[:])
    _split_sync_waits(nc)
    _NC_CACHE["nc"] = nc
    return nc


# ---------------------------------------------------------------------------
# Host entry point


def kernel(x: np.ndarray, **run_kwargs) -> np.ndarray:
    assert x.shape == (T, B, C, H, W) and x.dtype == np.float32
    nc = _build()
    xr = np.ascontiguousarray(x).reshape(T, B, C, HW)
    eye_np = np.eye(C, dtype=np.float32).astype(_BF16)
    eyed_np = (np.eye(C, dtype=np.float32) * np.float32(DECAY_R)).astype(
        np.float32)
    in_maps = []
    for m in range(N_CORES):
        xc = np.ascontiguousarray(
            xr[:, m * BS:(m + 1) * BS].transpose(0, 2, 1, 3)).reshape(
                T, C, F)
        # Dekker split: x = hi + lo with hi = bf16(x), lo = bf16(x - hi);
        # x - hi is exact in fp32, so the combined error is <= 2^-18 |x|
        hi = xc.astype(_BF16)
        lo = (xc - hi.astype(np.float32)).astype(_BF16)
        # pack per group: xhl[t, g] = [C, 2*GF] = [hi_g | lo_g]
        xhl = np.empty((T, G, C, 2 * GF), dtype=_BF16)
        for g in range(G):
            sl = slice(g * GF, (g + 1) * GF)
            xhl[:, g, :, :GF] = hi[:, :, sl]
            xhl[:, g, :, GF:] = lo[:, :, sl]
        in_maps.append({"xhl": xhl, "eye": eye_np, "eyed": eyed_np})
    res = run_bass_kernel_spmd(nc, in_maps, list(range(N_CORES)), **run_kwargs)
    full = np.empty((T, B, C, HW), np.float32)
    for m in range(N_CORES):
        s8 = np.asarray(res.results[m]["out_s"]).reshape(
            T, C, BS, HW).transpose(0, 2, 1, 3)
        # groups (= batches) 0-1 used the ACT relu encode, 2-3 the DVE one
        sa = s8[:, :2]
        da = (sa >= 126).astype(np.float32) - (sa == 0).astype(np.float32)
        sd = s8[:, 2:]
        dd = (sd >= 64).astype(np.float32) - (sd <= -64).astype(np.float32)
        full[:, m * BS:m * BS + 2] = da
        full[:, m * BS + 2:(m + 1) * BS] = dd
    if run_kwargs:
        kernel.last_results = res
    return full.reshape(T, B, C, H, W)


# revision 15
# speedup vs baseline: 1.0239x; 1.0239x over previous
"""LIF neuron (leaky integrate, bidirectional threshold fire, hard reset)
on 8 Trainium2 NeuronCores.

Math (per element, recurrence over T):
    u      = V*(1 - 1/tau) + x_t        (tau = 5/3  =>  decay ~= 0.4)
    out_t  = (u >= 1) - (u <= -1)               in {-1, 0, +1}
    V'     = u * (|u| < 1)                      (hard reset to 0)

Sharding: data-parallel over batch (axis 1), B=32 -> 4 per core; the
recurrence is only over T and elementwise over B,C,H,W, so no
communication is needed.

Engine split (the baseline ran the integrate as a DVE stt at 1x fp32
rate, saturating DVE at ~69us and ACT at ~66us while PE sat idle):
  PE   u = (decay*I)^T V + I^T x_hi + I^T x_lo    three accumulating
       identity matmuls per 512-col PSUM bank.  x is Dekker-split on
       the host into two bf16 halves (x = hi + lo exactly to 2^-18
       rel), so the x path runs at full PE rate with fp32-grade
       precision and the SAME HBM bytes as one fp32 tensor.  The V
       path uses fp32r (tf32-like, ~2^-12): V' is bounded by 1 and
       only survives one decay step, so the extra ~2e-5 error in u
       flips only O(1e3) of the 10.7M spikes (rel err ~1e-2 < 2e-2).
  ACT  a  = |u|                        activation(Abs), PSUM source
  DVE  V' = (a is_lt 1) mult u         scalar_tensor_tensor -> fp32r
  enc  s8: groups 0-1 on ACT  sat_i8(relu(62.5*u + 63)),
           groups 2-3 on DVE  sat_i8(max(u,-1)*63.5)
Host decode: ACT groups  spike = (s8 >= 126) - (s8 == 0)
             DVE groups  spike = (s8 >= 64) - (s8 <= -64)
HBM traffic per core: 16.8 MB in + 4.2 MB out.
"""

import numpy as np

import concourse.bass as bass
import concourse.tile as tile
from concourse import mybir
from concourse.alu_op_type import AluOpType
from concourse.bass_utils import run_bass_kernel_spmd

try:
    import ml_dtypes
    _BF16 = ml_dtypes.bfloat16
except ImportError:  # pragma: no cover
    _BF16 = None


def _split_sync_waits(nc):
    """This walrus build enforces the ISA limit of one sync wait per
    instruction (two for EventSemaphore), but Tile's sem-assigner freely
    attaches several. Hoist excess waits onto NoOps inserted just before the
    offending instruction on the same engine (waits are monotonic sem-ge, so
    order among them is irrelevant)."""
    ctr = 0
    for f in nc.m.functions:
        for bb in f.blocks:
            il = bb.instructions
            i = 0
            while i < len(il):
                inst = il[i]
                si = getattr(inst, "sync_info", None)
                if si is not None:
                    lim = 2 if isinstance(inst, mybir.InstEventSemaphore) else 1
                    waits = list(si.on_wait)
                    if len(waits) > lim:
                        inst.sync_info = mybir.SyncInfo(
                            on_wait=waits[:lim], on_update=list(si.on_update))
                        for w in waits[lim:]:
                            ctr += 1
                            nop = mybir.InstNoOp(
                                name=f"I-wsplit-{ctr}",
                                engine=inst.engine,
                                bass_nofuse=True,
                                sync_info=mybir.SyncInfo(
                                    on_wait=[w], on_update=[]),
                            )
                            nc.register_instruction(nop, overwrite=True)
                            il.insert(i, nop)
                            i += 1
                i += 1
    return ctr


# ---------------------------------------------------------------------------
# Problem shape (hardcoded per spec: x [T, B, C, H, W] = [8, 32, 128, 32, 32])
T, B, C, H, W = 8, 32, 128, 32, 32
HW = H * W                      # 1024
N_CORES = 8
BS = B // N_CORES               # 4 batches per core
F = BS * HW                     # 4096 free-dim elements per step
G = 4                           # pipeline groups per step (1 batch each)
GF = F // G                     # 1024 (= 2 PSUM banks)
# fp32r(0.4): nearest tf32-grid value to the reference decay; hardcoded so
# host, CoreSim and hardware all use the identical weight
DECAY_R = 0.4000244140625  # fp32r(0.4): V path runs in fp32r

ENC_SCALE = 62.5
ENC_BIAS = 63.0

F32 = mybir.dt.float32
F32R = mybir.dt.float32r
BF16 = mybir.dt.bfloat16
I8 = mybir.dt.int8
ABS = mybir.ActivationFunctionType.Abs
RELU = mybir.ActivationFunctionType.Relu

_NC_CACHE = {}


def _build():
    if "nc" in _NC_CACHE:
        return _NC_CACHE["nc"]
    nc = bass.Bass()
    # hi/lo bf16 halves packed per group: xhl[t, g] = [C, 2*GF] with
    # columns [0:GF] = bf16 hi, [GF:2*GF] = bf16 lo -> one DMA per group
    xhl = nc.declare_dram_parameter("xhl", [T, G, C, 2 * GF], BF16,
                                    isOutput=False)
    eye = nc.declare_dram_parameter("eye", [C, C], BF16, isOutput=False)
    eyed = nc.declare_dram_parameter("eyed", [C, C], F32R, isOutput=False)
    out_s = nc.declare_dram_parameter("out_s", [T, C, F], I8, isOutput=True)

    with tile.TileContext(nc) as tc:
        with (
            tc.tile_pool(name="xp", bufs=12) as xp,
            tc.tile_pool(name="vp", bufs=8) as vp,
            tc.tile_pool(name="ap", bufs=4) as ap,
            tc.tile_pool(name="ep", bufs=3) as ep,
            tc.tile_pool(name="wp", bufs=1) as wp,
            tc.tile_pool(name="pp", bufs=4,
                         space=bass.MemorySpace.PSUM) as pp,
        ):
            bt = wp.tile([C, 1], F32, tag="bias")
            nc.vector.memset(bt[:], ENC_BIAS)
            eye_t = wp.tile([C, C], BF16, tag="eye")
            nc.sync.dma_start(out=eye_t[:], in_=eye[:])
            eyed_t = wp.tile([C, C], F32R, tag="eyed")
            nc.sync.dma_start(out=eyed_t[:], in_=eyed[:])
            # preload the ACT table so the first real activation doesn't
            # pay the ~2.7us table load on the critical path
            warm = wp.tile([C, 1], F32, tag="warm")
            nc.scalar.activation(warm[:], bt[:], ABS)
            nc.scalar.activation(warm[:], bt[:], RELU)

            state = [None] * G
            for t in range(T):
                xts = []
                for g in range(G):
                    xg = xp.tile([C, 2 * GF], BF16, tag="xg")
                    nc.sync.dma_start(out=xg[:], in_=xhl[t][g])
                    xts.append(xg)
                # x matmuls first: they only need the DMA, so the PE queue
                # never stalls on the recurrence and the HAM stays warm
                ps = []
                for g in range(G):
                    p = pp.tile([C, GF], F32, tag="p")
                    xg = xts[g]
                    for h in range(2):
                        sl = slice(h * 512, (h + 1) * 512)
                        nc.tensor.matmul(
                            p[:, sl], eye_t[:],
                            xg[:, h * 512:(h + 1) * 512],
                            start=True, stop=False)
                        nc.tensor.matmul(
                            p[:, sl], eye_t[:],
                            xg[:, GF + h * 512:GF + (h + 1) * 512],
                            start=False, stop=(t == 0))
                    ps.append(p)
                # V matmuls accumulate last (they wait on the previous
                # step's reset)
                if t > 0:
                    for g in range(G):
                        for h in range(2):
                            sl = slice(h * 512, (h + 1) * 512)
                            nc.tensor.matmul(
                                ps[g][:, sl], eyed_t[:], state[g][:, sl],
                                start=False, stop=True)
                # per group: abs -> reset -> encode, so each PSUM bank pair
                # frees as early as possible (the next step's x matmuls are
                # queued on the PE waiting for it, and a >3us PE gap would
                # re-throttle the HAM to half rate)
                e = ep.tile([C, F], I8)
                for g in range(G):
                    sl = slice(g * GF, (g + 1) * GF)
                    if t < T - 1:
                        a = ap.tile([C, GF], F32)
                        nc.scalar.activation(a[:], ps[g][:], ABS)
                        v = vp.tile([C, GF], F32R, tag="v")
                        nc.vector.scalar_tensor_tensor(
                            v[:], a[:], 1.0, ps[g][:],
                            AluOpType.is_lt, AluOpType.mult)
                        state[g] = v
                    if g < 2:
                        nc.scalar.activation(e[:, sl], ps[g][:], RELU,
                                             bias=bt[:], scale=ENC_SCALE)
                    else:
                        nc.vector.tensor_scalar(
                            e[:, sl], ps[g][:], -1.0, 63.5,
                            AluOpType.max, AluOpType.mult)
                # store via the (otherwise idle) GpSimd queue: its issue
                # waits on the encodes, and on the Sync queue it would block
                # the next step's loads behind that wait
                nc.gpsimd.dma_start(out=out_s[t][:], in_=e[:])
    _split_sync_waits(nc)
    _NC_CACHE["nc"] = nc
    return nc


# ---------------------------------------------------------------------------
# Host entry point


def kernel(x: np.ndarray, **run_kwargs) -> np.ndarray:
    assert x.shape == (T, B, C, H, W) and x.dtype == np.float32
    nc = _build()
    xr = np.ascontiguousarray(x).reshape(T, B, C, HW)
    eye_np = np.eye(C, dtype=np.float32).astype(_BF16)
    eyed_np = (np.eye(C, dtype=np.float32) * np.float32(DECAY_R)).astype(
        np.float32)
    in_maps = []
    for m in range(N_CORES):
        xc = np.ascontiguousarray(
            xr[:, m * BS:(m + 1) * BS].transpose(0, 2, 1, 3)).reshape(
                T, C, F)
        # Dekker split: x = hi + lo with hi = bf16(x), lo = bf16(x - hi);
        # x - hi is exact in fp32, so the combined error is <= 2^-18 |x|
        hi = xc.astype(_BF16)
        lo = (xc - hi.astype(np.float32)).astype(_BF16)
        # pack per group: xhl[t, g] = [C, 2*GF] = [hi_g | lo_g]
        xhl = np.empty((T, G, C, 2 * GF), dtype=_BF16)
        for g in range(G):
            sl = slice(g * GF, (g + 1) * GF)
            xhl[:, g, :, :GF] = hi[:, :, sl]
            xhl[:, g, :, GF:] = lo[:, :, sl]
        in_maps.append({"xhl": xhl, "eye": eye_np, "eyed": eyed_np})
    res = run_bass_kernel_spmd(nc, in_maps, list(range(N_CORES)), **run_kwargs)
    full = np.empty((T, B, C, HW), np.float32)
    for m in range(N_CORES):
        s8 = np.asarray(res.results[m]["out_s"]).reshape(
            T, C, BS, HW).transpose(0, 2, 1, 3)
        # groups (= batches) 0-1 used the ACT relu encode, 2-3 the DVE one
        sa = s8[:, :2]
        da = (sa >= 126).astype(np.float32) - (sa == 0).astype(np.float32)
        sd = s8[:, 2:]
        dd = (sd >= 64).astype(np.float32) - (sd <= -64).astype(np.float32)
        full[:, m * BS:m * BS + 2] = da
        full[:, m * BS + 2:(m + 1) * BS] = dd
    if run_kwargs:
        kernel.last_results = res
    return full.reshape(T, B, C, H, W)
